# revision 51
# baseline (speedup 1.0000x reference)
"""MicrotubuleAttention TRN2 kernel, head-sharded across 8 NeuronCores.

Core c computes q-heads {2c, 2c+1} with kv-head c//2 and its slice of the
output projection; the host sums the 8 partial projections.

Key structure (single activation-table kernel):
 - Projections in [t, d] layout; RoPE via free-dim half-shifts; Q/K/A-out
   transposed to [d, t] on the PE for the attention matmuls.
 - Scores computed TRANSPOSED per 128-block: S^T[tk, tq] so exp() emits P^T
   directly (no per-block transpose of the softmax matrix).
 - Bias (gate*sigmoid(lowrank) + c_h*(j - row_max)) is prefilled into PSUM by
   the DVE before the QK matmul accumulates on top; sigmoid is computed as
   0.5*tanh(m/2)+0.5 so tanh/exp share one activation table (no reloads).
 - The shifted c_h*(j - 128i - 127) bias keeps scores <= ~7, so softmax runs
   without a max pass; the row sum comes from a ones-column appended to V and
   normalization uses the DVE fast reciprocal.
"""
import numpy as np

D_MODEL = 2048
N_HEADS = 16
D_HEAD = 128
MAX_SEQ_LEN = 4096
RANK = 32
ROPE_BASE = 10000.0
T = 2048
N_CORES = 8
HPC = 2                  # q heads per core
P = 128
NT = T // P              # 16 row tiles
ND = D_MODEL // P        # 16 d-model chunks
SD = float(np.sqrt(D_HEAD))
CH = 8                   # j-blocks per PSUM chunk (8*128 = 1024 cols)
NEG = -1.0e30


def _build_kernel():
    import concourse.bass as bass
    import concourse.mybir as mybir
    import concourse.tile as tile
    from concourse import bacc
    from concourse.masks import make_identity
    from contextlib import ExitStack

    f32 = mybir.dt.float32
    bf16 = mybir.dt.bfloat16
    AF = mybir.ActivationFunctionType
    ALU = mybir.AluOpType

    nc = bacc.Bacc("TRN2", target_bir_lowering=False, debug=False,
                   num_devices=N_CORES)

    xt = nc.dram_tensor("xt", [P, ND * T], bf16, kind="ExternalInput")
    wqkv = nc.dram_tensor("wqkv", [P, ND * 512], bf16, kind="ExternalInput")
    wab = nc.dram_tensor("wab", [P, ND * 2 * RANK], bf16, kind="ExternalInput")
    cosd = nc.dram_tensor("cosd", [P, NT * P], bf16, kind="ExternalInput")
    sind = nc.dram_tensor("sind", [P, NT * P], bf16, kind="ExternalInput")
    wo = nc.dram_tensor("wo", [P, HPC * D_MODEL], bf16, kind="ExternalInput")
    fh = nc.dram_tensor("fh", [P, HPC], f32, kind="ExternalInput")
    c1b = nc.dram_tensor("c1b", [P, HPC * NT], bf16, kind="ExternalInput")
    out = nc.dram_tensor("out", [T, D_MODEL], bf16, kind="ExternalOutput")

    with tile.TileContext(nc) as tc, ExitStack() as ctx:
        con = ctx.enter_context(tc.tile_pool(name="con", bufs=1))
        work = ctx.enter_context(tc.tile_pool(name="work", bufs=3))
        wk3 = ctx.enter_context(tc.tile_pool(name="wk3", bufs=4))
        small = ctx.enter_context(tc.tile_pool(name="small", bufs=4))
        ps_pq = ctx.enter_context(tc.tile_pool(name="ps_pq", bufs=1, space="PSUM"))
        ps_ms = ctx.enter_context(tc.tile_pool(name="ps_ms", bufs=2, space="PSUM"))
        ps_ot = ctx.enter_context(tc.tile_pool(name="ps_ot", bufs=1, space="PSUM"))
        ps_tp = ctx.enter_context(tc.tile_pool(name="ps_tp", bufs=1, space="PSUM"))
        ps_op = ctx.enter_context(tc.tile_pool(name="ps_op", bufs=1, space="PSUM"))

        # ---- constants / inputs (DMA order = criticality) ----
        # wqkv + the first 512 t-cols of xt stream in d-chunk groups so the
        # first projection chain starts within a few us.
        wqkv_sb = con.tile([P, ND, 512], bf16)
        xt_sb = con.tile([P, ND, T], bf16)
        wab_sb = con.tile([P, ND, 2 * RANK], bf16)
        nc.sync.dma_start(out=wab_sb[:], in_=wab[:, :])
        for g in range(4):
            dsl = slice(g * 4, (g + 1) * 4)
            nc.sync.dma_start(out=wqkv_sb[:, dsl],
                              in_=wqkv[:, g * 4 * 512:(g + 1) * 4 * 512])
            xs = xt[:, 0:1]
            src = bass.AP(tensor=xs.tensor, offset=xs.offset + g * 4 * T,
                          ap=[xs.ap[0], [T, 4], [1, 512]])
            nc.sync.dma_start(out=xt_sb[:, dsl, 0:512], in_=src)
        cos_sb = con.tile([P, NT, P], bf16)
        nc.sync.dma_start(out=cos_sb[:], in_=cosd[:, :])
        sin_sb = con.tile([P, NT, P], bf16)
        nc.sync.dma_start(out=sin_sb[:], in_=sind[:, :])
        fh_sb = con.tile([P, HPC], f32)
        nc.sync.dma_start(out=fh_sb[:], in_=fh[:, :])
        c1b_sb = con.tile([P, HPC, NT], bf16)
        nc.sync.dma_start(out=c1b_sb[:], in_=c1b[:, :])
        wo_sb = con.tile([P, HPC, D_MODEL], bf16)
        for g in range(3):
            c0 = 512 * (g + 1)
            xs = xt[:, 0:1]
            src = bass.AP(tensor=xs.tensor, offset=xs.offset + c0,
                          ap=[xs.ap[0], [T, ND], [1, 512]])
            nc.sync.dma_start(out=xt_sb[:, :, c0:c0 + 512], in_=src)
            if g == 0:
                nc.sync.dma_start(out=wo_sb[:], in_=wo[:, :])
        ident = con.tile([P, P], bf16)
        make_identity(nc, ident)

        qkt = con.tile([P, 3, T], bf16)          # q0^T q1^T k^T in [d, t]
        v_sb = con.tile([P, NT, 132], bf16)      # V [tk, d] + ones col @128
        abt_sb = con.tile([2 * RANK, T], bf16)   # [A^T; B^T] rows 0:32 / 32:64
        bt_sb = con.tile([RANK, T], bf16)        # B^T relocated to partition 0
        nc.vector.memset(v_sb[:, :, 128:129], 1.0)

        def view3(ap, n, w):
            """reshape trailing free dim of a 2-D AP slice into [n, w]"""
            return bass.AP(tensor=ap.tensor, offset=ap.offset,
                           ap=[ap.ap[0], [ap.ap[1][0] * w, n],
                               [ap.ap[1][0], w]])

        def bcast(ap, w):
            """[P, n] AP -> [P, n, w] with stride-0 last dim"""
            return bass.AP(tensor=ap.tensor, offset=ap.offset,
                           ap=[ap.ap[0], ap.ap[1], [0, w]])

        def bcast_mid(ap, n):
            """[P, w] AP -> [P, n, w] with stride-0 middle dim"""
            return bass.AP(tensor=ap.tensor, offset=ap.offset,
                           ap=[ap.ap[0], [0, n], ap.ap[1]])

        def ab_chunk(c):
            csl = slice(c * 512, (c + 1) * 512)
            pa = ps_pq.tile([P, 512], mybir.dt.float32, tag="pq")
            for d in range(ND):
                nc.tensor.matmul(pa[0:2 * RANK, :], wab_sb[:, d],
                                 xt_sb[:, d, csl],
                                 start=(d == 0), stop=(d == ND - 1))
            nc.scalar.copy(abt_sb[:, csl], pa[0:2 * RANK, :])
            # relocate B^T rows to partition base 0 (matmul operands must share
            # a base partition; engines can't shift partitions, DMA can)
            nc.sync.dma_start(out=bt_sb[:, csl], in_=abt_sb[RANK:2 * RANK, csl])

        def proj(i):
            tsl = slice(i * P, (i + 1) * P)
            pq = ps_pq.tile([P, 512], mybir.dt.float32, tag="pq")
            for d in range(ND):
                nc.tensor.matmul(pq[:], xt_sb[:, d, tsl], wqkv_sb[:, d],
                                 start=(d == 0), stop=(d == ND - 1))
            qkc = work.tile([P, 4, P], bf16, tag="qkc")
            nc.scalar.copy(qkc[:], pq[:])
            nc.gpsimd.tensor_copy(v_sb[:, i, 0:128], qkc[:, 3])
            # RoPE in [t, d]: half-shifted reads, sign baked into sin table
            ro = work.tile([P, 3, P], bf16, tag="ro")
            nc.vector.tensor_tensor(
                ro[:, :, 0:64], qkc[:, 0:3, 64:128],
                bcast_mid(sin_sb[:, i, 0:64], 3), op=ALU.mult)
            nc.vector.tensor_tensor(
                ro[:, :, 64:128], qkc[:, 0:3, 0:64],
                bcast_mid(sin_sb[:, i, 64:128], 3), op=ALU.mult)
            qc = work.tile([P, 3, P], bf16, tag="qc")
            nc.vector.tensor_tensor(
                qc[:], qkc[:, 0:3], bcast_mid(cos_sb[:, i, :], 3), op=ALU.mult)
            roped = work.tile([P, 3, P], bf16, tag="roped")
            nc.gpsimd.tensor_tensor(roped[:], qc[:], ro[:], op=ALU.add)
            tp_ = ps_tp.tile([P, 384], bf16, tag="tp")
            for hh in range(3):
                nc.tensor.transpose(tp_[:, hh * P:(hh + 1) * P],
                                    roped[:, hh], ident[:])
            nc.scalar.copy(qkt[:, :, tsl], tp_[:])

        def sview(flat_ap, h, nb):
            """[128, 1024] psum slice -> [128, nb, 128] picking head h out of
            interleaved 256-wide blocks"""
            base = flat_ap  # AP over sp_[:, h*128 : h*128+128]
            return bass.AP(tensor=base.tensor, offset=base.offset,
                           ap=[base.ap[0], [256, nb], base.ap[1]])

        th_tiles = {}

        def mth(i):
            tsl = slice(i * P, (i + 1) * P)
            nb_all = i + 1
            th = work.tile([P, NT, P], bf16, tag="th")
            th_tiles[i] = th
            for j0 in range(0, nb_all, CH):
                nb = min(CH, nb_all - j0)
                mp_ = ps_ms.tile([P, CH * P], mybir.dt.float32, tag="ms")
                for jj in range(nb):
                    j = j0 + jj
                    nc.tensor.matmul(mp_[:, jj * P:(jj + 1) * P],
                                     bt_sb[:, j * P:(j + 1) * P],
                                     abt_sb[0:RANK, tsl], start=True, stop=True,
                                     skip_group_check=True)
                nc.scalar.activation(th[:, j0:j0 + nb], mp_[:, 0:nb * P],
                                     AF.Tanh, scale=0.5)

        def attn(i):
            tsl = slice(i * P, (i + 1) * P)
            nb_all = i + 1
            th = th_tiles.pop(i)
            # merged-head scores: one matmul per (i, j) block, rhs = [q0|q1];
            # pm keeps the head-interleaved layout [tk_p, j, h, tq] so one exp
            # instruction covers both heads (per-partition bias lives in c1be)
            pm = work.tile([P, NT, HPC, P], bf16, tag="pm")
            for j0 in range(0, nb_all, 4):
                nb = min(4, nb_all - j0)
                sp_ = ps_ms.tile([P, CH * P], mybir.dt.float32, tag="ms")
                for jj in range(nb):
                    j = j0 + jj
                    nc.tensor.matmul(sp_[:, jj * 256:(jj + 1) * 256],
                                     qkt[:, 2, j * P:(j + 1) * P],
                                     qkt[:, 0:2, tsl],
                                     start=True, stop=True,
                                     skip_group_check=True)
                nc.scalar.activation(pm[:, j0:j0 + nb], sp_[:, 0:nb * 256],
                                     AF.Exp, scale=1.0 / SD)
            aoutT = wk3.tile([P, HPC, P], bf16, tag="aoutT")
            b0 = NT - 1 - i
            for h in range(HPC):
                pmh = bass.AP(tensor=pm[:].tensor,
                              offset=pm[:].offset + h * P,
                              ap=[pm[:].ap[0], [HPC * P, nb_all], [1, P]])
                # fold gate*sigmoid + per-block decay + exp bias into P^T:
                #   pm *= c1be[p, b] * (th + 1 + 2/g)
                # (chunked on the last tiles so AV can start early in the tail)
                FCH = CH
                tmp = work.tile([P, NT, P], bf16, tag="ftmp")
                for f0 in range(0, nb_all, FCH):
                    fn = min(FCH, nb_all - f0)
                    nc.vector.tensor_scalar_add(tmp[:, f0:f0 + fn],
                                                th[:, f0:f0 + fn],
                                                fh_sb[:, h:h + 1])
                    nc.vector.tensor_tensor(
                        tmp[:, f0:f0 + fn], tmp[:, f0:f0 + fn],
                        bcast(c1b_sb[:, h, b0 + f0:b0 + f0 + fn], P),
                        op=ALU.mult)
                    nc.vector.tensor_tensor(pm[:, f0:f0 + fn, h],
                                            pm[:, f0:f0 + fn, h],
                                            tmp[:, f0:f0 + fn], op=ALU.mult)
                # causal mask on the diagonal block: keep tk(p) <= tq(f)
                nc.gpsimd.affine_select(out=pm[:, i, h], in_=pm[:, i, h],
                                        pattern=[[1, P]], compare_op=ALU.is_ge,
                                        fill=0.0, base=0, channel_multiplier=-1)
                ot_ = ps_ot.tile([P, 132], mybir.dt.float32, tag="ot")
                for j in range(nb_all):
                    nc.tensor.matmul(ot_[:, 0:129], pm[:, j, h],
                                     v_sb[:, j, 0:129],
                                     start=(j == 0), stop=(j == nb_all - 1))
                rec = small.tile([P, 1], mybir.dt.float32, tag="rec")
                nc.vector.reciprocal_approx_fast(out=rec[:], in_=ot_[:, 128:129])
                aout = work.tile([P, P], bf16, tag="aout")
                nc.vector.tensor_scalar_mul(aout[:], ot_[:, 0:128], rec[:])
                tp2 = ps_tp.tile([P, 384], bf16, tag="tp")
                nc.tensor.transpose(tp2[:, 0:128], aout[:], ident[:])
                nc.scalar.copy(aoutT[:, h], tp2[:, 0:128])
            # output projection for this row tile
            ob = work.tile([P, D_MODEL], bf16, tag="ob")
            for mch in range(4):
                po = ps_op.tile([P, 512], mybir.dt.float32, tag="po")
                for h in range(HPC):
                    nc.tensor.matmul(po[:], aoutT[:, h],
                                     wo_sb[:, h, mch * 512:(mch + 1) * 512],
                                     start=(h == 0), stop=(h == HPC - 1))
                if mch % 2 == 0:
                    nc.scalar.copy(ob[:, mch * 512:(mch + 1) * 512], po[:])
                else:
                    nc.vector.tensor_copy(ob[:, mch * 512:(mch + 1) * 512],
                                          po[:])
            nc.sync.dma_start(out=out[tsl, :], in_=ob[:])

        for i in range(NT):
            proj(i)
            if i % 4 == 0:
                ab_chunk(i // 4)
            mth(i)
            if i > 0:
                attn(i - 1)
        attn(NT - 1)

    nc.compile()
    return nc


_NC_CACHE = None


def kernel(**inputs):
    global _NC_CACHE
    import ml_dtypes
    bf = ml_dtypes.bfloat16

    x = np.asarray(inputs["x"])
    Wq = np.asarray(inputs["Wq"]); Wk = np.asarray(inputs["Wk"])
    Wv = np.asarray(inputs["Wv"]); Wo = np.asarray(inputs["Wo"])
    pol_dir = np.asarray(inputs["pol_dir"]); pol_WA = np.asarray(inputs["pol_WA"])
    pol_WB = np.asarray(inputs["pol_WB"]); pol_gate = np.asarray(inputs["pol_gate"])
    gtp_gamma = np.asarray(inputs["gtp_gamma"])
    assert x.shape == (1, T, D_MODEL)

    pol = np.clip(pol_dir.astype(np.float64), -1.0, 1.0)
    gamma = np.maximum(np.log1p(np.exp(gtp_gamma.astype(np.float64))), 1e-6)
    c_h = (pol / float(MAX_SEQ_LEN) + gamma)                      # [16]
    gate = 1.0 / (1.0 + np.exp(-pol_gate.astype(np.float64)))     # [16]

    # xt[p, d*T + t] = x[t, 128d + p]
    xt = np.ascontiguousarray(
        x[0].T.reshape(ND, P, T).transpose(1, 0, 2).reshape(P, ND * T)
    ).astype(bf)

    # RoPE tables in [t(part), tile, d]: cos[t, d]; sin'[t, d] has the
    # rotate-half sign baked in (negative for d < 64).
    inv_freq = 1.0 / (ROPE_BASE ** (np.arange(0, D_HEAD, 2, dtype=np.float64) / D_HEAD))
    tpos = np.arange(T, dtype=np.float64)
    ang = tpos[:, None] * inv_freq[None, :]                       # [T, 64]
    cos_td = np.concatenate([np.cos(ang), np.cos(ang)], 1)        # [T, 128]
    sin_td = np.concatenate([-np.sin(ang), np.sin(ang)], 1)
    cosd = cos_td.reshape(NT, P, P).transpose(1, 0, 2).reshape(P, NT * P).astype(bf)
    sind = sin_td.reshape(NT, P, P).transpose(1, 0, 2).reshape(P, NT * P).astype(bf)

    pp = np.arange(P, dtype=np.float64)

    if _NC_CACHE is None:
        _NC_CACHE = _build_kernel()

    in_maps = []
    for c in range(N_CORES):
        heads = [2 * c, 2 * c + 1]
        kv = c // 2
        # wqkv[p, d*512 + [q0|q1|k|v]]
        wq_c = Wq[:, heads[0] * D_HEAD:(heads[0] + 1) * D_HEAD]
        wq_c2 = Wq[:, heads[1] * D_HEAD:(heads[1] + 1) * D_HEAD]
        wk_c = Wk[:, kv * D_HEAD:(kv + 1) * D_HEAD]
        wv_c = Wv[:, kv * D_HEAD:(kv + 1) * D_HEAD]
        wqkv_full = np.concatenate([wq_c, wq_c2, wk_c, wv_c], axis=1)  # [2048, 512]
        wqkv_h = wqkv_full.reshape(ND, P, 512).transpose(1, 0, 2).reshape(P, ND * 512)
        wab_full = np.concatenate([pol_WA, pol_WB], axis=1)            # [2048, 64]
        wab_h = wab_full.reshape(ND, P, 2 * RANK).transpose(1, 0, 2).reshape(P, -1)
        # wo[p, h*2048 + m] = Wo[head*128 + p, m]
        wo_h = np.concatenate(
            [Wo[h * D_HEAD:(h + 1) * D_HEAD, :] for h in heads], axis=1)
        # fh = 1 + 2/g;  c1be[p, s, b] = (g/2)*exp(-128c(15-b))*exp(c(p-127))
        # (the last factor is the per-key exp bias, folded into the table)
        fh_h = np.zeros((P, HPC), dtype=np.float64)
        c1b_h = np.zeros((P, HPC, NT), dtype=np.float64)
        for s, h in enumerate(heads):
            bb = np.arange(NT, dtype=np.float64)
            fh_h[:, s] = 1.0 + 2.0 / gate[h]
            c1b_h[:, s, :] = ((gate[h] / 2.0)
                              * np.exp(-128.0 * c_h[h] * (15.0 - bb[None, :]))
                              * np.exp(c_h[h] * (pp[:, None] - 127.0)))
        in_maps.append({
            "xt": xt,
            "wqkv": np.ascontiguousarray(wqkv_h).astype(bf),
            "wab": np.ascontiguousarray(wab_h).astype(bf),
            "cosd": cosd, "sind": sind,
            "wo": np.ascontiguousarray(wo_h).astype(bf),
            "fh": fh_h.astype(np.float32),
            "c1b": np.ascontiguousarray(c1b_h.reshape(P, HPC * NT)).astype(bf),
        })

    from concourse.bass_utils import run_bass_kernel_spmd
    res = run_bass_kernel_spmd(_NC_CACHE, in_maps, core_ids=list(range(N_CORES)))
    import sys as _sys
    _sys.modules[__name__]._LAST_RES = res
    total = np.zeros((T, D_MODEL), dtype=np.float32)
    for c in range(N_CORES):
        total += res.results[c]["out"].astype(np.float32)
    return total[None, :, :]


# revision 52
# speedup vs baseline: 1.0073x; 1.0073x over previous
"""MicrotubuleAttention TRN2 kernel, head-sharded across 8 NeuronCores.

Core c computes q-heads {2c, 2c+1} with kv-head c//2 and its slice of the
output projection; the host sums the 8 partial projections.

Key structure (single activation-table kernel):
 - Projections in [t, d] layout; RoPE via free-dim half-shifts; Q/K/A-out
   transposed to [d, t] on the PE for the attention matmuls.
 - Scores computed TRANSPOSED per 128-block: S^T[tk, tq] so exp() emits P^T
   directly (no per-block transpose of the softmax matrix).
 - Bias (gate*sigmoid(lowrank) + c_h*(j - row_max)) is prefilled into PSUM by
   the DVE before the QK matmul accumulates on top; sigmoid is computed as
   0.5*tanh(m/2)+0.5 so tanh/exp share one activation table (no reloads).
 - The shifted c_h*(j - 128i - 127) bias keeps scores <= ~7, so softmax runs
   without a max pass; the row sum comes from a ones-column appended to V and
   normalization uses the DVE fast reciprocal.
"""
import numpy as np

D_MODEL = 2048
N_HEADS = 16
D_HEAD = 128
MAX_SEQ_LEN = 4096
RANK = 32
ROPE_BASE = 10000.0
T = 2048
N_CORES = 8
HPC = 2                  # q heads per core
P = 128
NT = T // P              # 16 row tiles
ND = D_MODEL // P        # 16 d-model chunks
SD = float(np.sqrt(D_HEAD))
CH = 8                   # j-blocks per PSUM chunk (8*128 = 1024 cols)
NEG = -1.0e30


def _build_kernel():
    import concourse.bass as bass
    import concourse.mybir as mybir
    import concourse.tile as tile
    from concourse import bacc
    from concourse.masks import make_identity
    from contextlib import ExitStack

    f32 = mybir.dt.float32
    bf16 = mybir.dt.bfloat16
    AF = mybir.ActivationFunctionType
    ALU = mybir.AluOpType

    nc = bacc.Bacc("TRN2", target_bir_lowering=False, debug=False,
                   num_devices=N_CORES)

    xt = nc.dram_tensor("xt", [P, ND * T], bf16, kind="ExternalInput")
    wqkv = nc.dram_tensor("wqkv", [P, ND * 512], bf16, kind="ExternalInput")
    wab = nc.dram_tensor("wab", [P, ND * 2 * RANK], bf16, kind="ExternalInput")
    cosd = nc.dram_tensor("cosd", [P, NT * P], bf16, kind="ExternalInput")
    sind = nc.dram_tensor("sind", [P, NT * P], bf16, kind="ExternalInput")
    wo = nc.dram_tensor("wo", [P, HPC * D_MODEL], bf16, kind="ExternalInput")
    fh = nc.dram_tensor("fh", [P, HPC], f32, kind="ExternalInput")
    c1b = nc.dram_tensor("c1b", [P, HPC * NT], bf16, kind="ExternalInput")
    out = nc.dram_tensor("out", [T, D_MODEL], bf16, kind="ExternalOutput")

    with tile.TileContext(nc) as tc, ExitStack() as ctx:
        con = ctx.enter_context(tc.tile_pool(name="con", bufs=1))
        work = ctx.enter_context(tc.tile_pool(name="work", bufs=3))
        wk3 = ctx.enter_context(tc.tile_pool(name="wk3", bufs=4))
        small = ctx.enter_context(tc.tile_pool(name="small", bufs=4))
        ps_pq = ctx.enter_context(tc.tile_pool(name="ps_pq", bufs=1, space="PSUM"))
        ps_ms = ctx.enter_context(tc.tile_pool(name="ps_ms", bufs=2, space="PSUM"))
        ps_ot = ctx.enter_context(tc.tile_pool(name="ps_ot", bufs=1, space="PSUM"))
        ps_tp = ctx.enter_context(tc.tile_pool(name="ps_tp", bufs=1, space="PSUM"))
        ps_op = ctx.enter_context(tc.tile_pool(name="ps_op", bufs=1, space="PSUM"))

        # ---- constants / inputs (DMA order = criticality) ----
        # wqkv + the first 512 t-cols of xt stream in d-chunk groups so the
        # first projection chain starts within a few us.
        wqkv_sb = con.tile([P, ND, 512], bf16)
        xt_sb = con.tile([P, ND, T], bf16)
        wab_sb = con.tile([P, ND, 2 * RANK], bf16)
        nc.sync.dma_start(out=wab_sb[:], in_=wab[:, :])
        for g in range(4):
            dsl = slice(g * 4, (g + 1) * 4)
            nc.sync.dma_start(out=wqkv_sb[:, dsl],
                              in_=wqkv[:, g * 4 * 512:(g + 1) * 4 * 512])
            xs = xt[:, 0:1]
            src = bass.AP(tensor=xs.tensor, offset=xs.offset + g * 4 * T,
                          ap=[xs.ap[0], [T, 4], [1, 512]])
            nc.sync.dma_start(out=xt_sb[:, dsl, 0:512], in_=src)
        cos_sb = con.tile([P, NT, P], bf16)
        nc.sync.dma_start(out=cos_sb[:], in_=cosd[:, :])
        sin_sb = con.tile([P, NT, P], bf16)
        nc.sync.dma_start(out=sin_sb[:], in_=sind[:, :])
        fh_sb = con.tile([P, HPC], f32)
        nc.sync.dma_start(out=fh_sb[:], in_=fh[:, :])
        c1b_sb = con.tile([P, HPC, NT], bf16)
        nc.sync.dma_start(out=c1b_sb[:], in_=c1b[:, :])
        wo_sb = con.tile([P, HPC, D_MODEL], bf16)
        for g in range(3):
            c0 = 512 * (g + 1)
            xs = xt[:, 0:1]
            src = bass.AP(tensor=xs.tensor, offset=xs.offset + c0,
                          ap=[xs.ap[0], [T, ND], [1, 512]])
            nc.sync.dma_start(out=xt_sb[:, :, c0:c0 + 512], in_=src)
            if g == 0:
                nc.sync.dma_start(out=wo_sb[:], in_=wo[:, :])
        ident = con.tile([P, P], bf16)
        make_identity(nc, ident)

        qkt = con.tile([P, 3, T], bf16)          # q0^T q1^T k^T in [d, t]
        v_sb = con.tile([P, NT, 132], bf16)      # V [tk, d] + ones col @128
        abt_sb = con.tile([2 * RANK, T], bf16)   # [A^T; B^T] rows 0:32 / 32:64
        bt_sb = con.tile([RANK, T], bf16)        # B^T relocated to partition 0
        nc.vector.memset(v_sb[:, :, 128:129], 1.0)

        def view3(ap, n, w):
            """reshape trailing free dim of a 2-D AP slice into [n, w]"""
            return bass.AP(tensor=ap.tensor, offset=ap.offset,
                           ap=[ap.ap[0], [ap.ap[1][0] * w, n],
                               [ap.ap[1][0], w]])

        def bcast(ap, w):
            """[P, n] AP -> [P, n, w] with stride-0 last dim"""
            return bass.AP(tensor=ap.tensor, offset=ap.offset,
                           ap=[ap.ap[0], ap.ap[1], [0, w]])

        def bcast_mid(ap, n):
            """[P, w] AP -> [P, n, w] with stride-0 middle dim"""
            return bass.AP(tensor=ap.tensor, offset=ap.offset,
                           ap=[ap.ap[0], [0, n], ap.ap[1]])

        def ab_chunk(c):
            csl = slice(c * 512, (c + 1) * 512)
            pa = ps_pq.tile([P, 512], mybir.dt.float32, tag="pq")
            for d in range(ND):
                nc.tensor.matmul(pa[0:2 * RANK, :], wab_sb[:, d],
                                 xt_sb[:, d, csl],
                                 start=(d == 0), stop=(d == ND - 1))
            nc.scalar.copy(abt_sb[:, csl], pa[0:2 * RANK, :])
            # relocate B^T rows to partition base 0 (matmul operands must share
            # a base partition; engines can't shift partitions, DMA can)
            nc.sync.dma_start(out=bt_sb[:, csl], in_=abt_sb[RANK:2 * RANK, csl])

        def proj(i):
            tsl = slice(i * P, (i + 1) * P)
            pq = ps_pq.tile([P, 512], mybir.dt.float32, tag="pq")
            for d in range(ND):
                nc.tensor.matmul(pq[:], xt_sb[:, d, tsl], wqkv_sb[:, d],
                                 start=(d == 0), stop=(d == ND - 1))
            qkc = work.tile([P, 4, P], bf16, tag="qkc")
            nc.scalar.copy(qkc[:], pq[:])
            nc.gpsimd.tensor_copy(v_sb[:, i, 0:128], qkc[:, 3])
            # RoPE in [t, d]: half-shifted reads, sign baked into sin table
            ro = work.tile([P, 3, P], bf16, tag="ro")
            nc.vector.tensor_tensor(
                ro[:, :, 0:64], qkc[:, 0:3, 64:128],
                bcast_mid(sin_sb[:, i, 0:64], 3), op=ALU.mult)
            nc.vector.tensor_tensor(
                ro[:, :, 64:128], qkc[:, 0:3, 0:64],
                bcast_mid(sin_sb[:, i, 64:128], 3), op=ALU.mult)
            qc = work.tile([P, 3, P], bf16, tag="qc")
            nc.vector.tensor_tensor(
                qc[:], qkc[:, 0:3], bcast_mid(cos_sb[:, i, :], 3), op=ALU.mult)
            roped = work.tile([P, 3, P], bf16, tag="roped")
            nc.gpsimd.tensor_tensor(roped[:], qc[:], ro[:], op=ALU.add)
            tp_ = ps_tp.tile([P, 384], bf16, tag="tp")
            for hh in range(3):
                nc.tensor.transpose(tp_[:, hh * P:(hh + 1) * P],
                                    roped[:, hh], ident[:])
            nc.scalar.copy(qkt[:, :, tsl], tp_[:])

        def sview(flat_ap, h, nb):
            """[128, 1024] psum slice -> [128, nb, 128] picking head h out of
            interleaved 256-wide blocks"""
            base = flat_ap  # AP over sp_[:, h*128 : h*128+128]
            return bass.AP(tensor=base.tensor, offset=base.offset,
                           ap=[base.ap[0], [256, nb], base.ap[1]])

        th_tiles = {}

        def mth(i):
            tsl = slice(i * P, (i + 1) * P)
            nb_all = i + 1
            th = work.tile([P, NT, P], bf16, tag="th")
            th_tiles[i] = th
            for j0 in range(0, nb_all, CH):
                nb = min(CH, nb_all - j0)
                mp_ = ps_ms.tile([P, CH * P], mybir.dt.float32, tag="ms")
                for jj in range(nb):
                    j = j0 + jj
                    nc.tensor.matmul(mp_[:, jj * P:(jj + 1) * P],
                                     bt_sb[:, j * P:(j + 1) * P],
                                     abt_sb[0:RANK, tsl], start=True, stop=True,
                                     skip_group_check=True)
                nc.scalar.activation(th[:, j0:j0 + nb], mp_[:, 0:nb * P],
                                     AF.Tanh, scale=0.5)

        def attn(i):
            tsl = slice(i * P, (i + 1) * P)
            nb_all = i + 1
            th = th_tiles.pop(i)
            # merged-head scores: one matmul per (i, j) block, rhs = [q0|q1];
            # pm keeps the head-interleaved layout [tk_p, j, h, tq] so one exp
            # instruction covers both heads (per-partition bias lives in c1be)
            pm = work.tile([P, NT, HPC, P], bf16, tag="pm")
            for j0 in range(0, nb_all, 4):
                nb = min(4, nb_all - j0)
                sp_ = ps_ms.tile([P, CH * P], mybir.dt.float32, tag="ms")
                for jj in range(nb):
                    j = j0 + jj
                    nc.tensor.matmul(sp_[:, jj * 256:(jj + 1) * 256],
                                     qkt[:, 2, j * P:(j + 1) * P],
                                     qkt[:, 0:2, tsl],
                                     start=True, stop=True,
                                     skip_group_check=True)
                nc.scalar.activation(pm[:, j0:j0 + nb], sp_[:, 0:nb * 256],
                                     AF.Exp, scale=1.0 / SD)
            aoutT = wk3.tile([P, HPC, P], bf16, tag="aoutT")
            b0 = NT - 1 - i
            for h in range(HPC):
                pmh = bass.AP(tensor=pm[:].tensor,
                              offset=pm[:].offset + h * P,
                              ap=[pm[:].ap[0], [HPC * P, nb_all], [1, P]])
                # fold gate*sigmoid + per-block decay + exp bias into P^T:
                #   pm *= c1be[p, b] * (th + 1 + 2/g)
                # (chunked on the last tiles so AV can start early in the tail)
                FCH = CH
                tmp = work.tile([P, NT, P], bf16, tag="ftmp")
                for f0 in range(0, nb_all, FCH):
                    fn = min(FCH, nb_all - f0)
                    nc.vector.tensor_scalar_add(tmp[:, f0:f0 + fn],
                                                th[:, f0:f0 + fn],
                                                fh_sb[:, h:h + 1])
                    nc.vector.tensor_tensor(
                        tmp[:, f0:f0 + fn], tmp[:, f0:f0 + fn],
                        bcast(c1b_sb[:, h, b0 + f0:b0 + f0 + fn], P),
                        op=ALU.mult)
                    nc.vector.tensor_tensor(pm[:, f0:f0 + fn, h],
                                            pm[:, f0:f0 + fn, h],
                                            tmp[:, f0:f0 + fn], op=ALU.mult)
                # causal mask on the diagonal block: keep tk(p) <= tq(f)
                nc.gpsimd.affine_select(out=pm[:, i, h], in_=pm[:, i, h],
                                        pattern=[[1, P]], compare_op=ALU.is_ge,
                                        fill=0.0, base=0, channel_multiplier=-1)
                ot_ = ps_ot.tile([P, 132], mybir.dt.float32, tag="ot")
                for j in range(nb_all):
                    nc.tensor.matmul(ot_[:, 0:129], pm[:, j, h],
                                     v_sb[:, j, 0:129],
                                     start=(j == 0), stop=(j == nb_all - 1))
                rec = small.tile([P, 1], mybir.dt.float32, tag="rec")
                nc.vector.reciprocal_approx_fast(out=rec[:], in_=ot_[:, 128:129])
                aout = work.tile([P, P], bf16, tag="aout")
                nc.vector.tensor_scalar_mul(aout[:], ot_[:, 0:128], rec[:])
                tp2 = ps_tp.tile([P, 384], bf16, tag="tp")
                nc.tensor.transpose(tp2[:, 0:128], aout[:], ident[:])
                nc.scalar.copy(aoutT[:, h], tp2[:, 0:128])
            # output projection for this row tile
            ob = work.tile([P, D_MODEL], bf16, tag="ob")
            for mch in range(4):
                po = ps_op.tile([P, 512], mybir.dt.float32, tag="po")
                for h in range(HPC):
                    nc.tensor.matmul(po[:], aoutT[:, h],
                                     wo_sb[:, h, mch * 512:(mch + 1) * 512],
                                     start=(h == 0), stop=(h == HPC - 1))
                if mch % 2 == 0:
                    nc.scalar.copy(ob[:, mch * 512:(mch + 1) * 512], po[:])
                else:
                    nc.vector.tensor_copy(ob[:, mch * 512:(mch + 1) * 512],
                                          po[:])
                    nc.sync.dma_start(
                        out=out[tsl, (mch - 1) * 512:(mch + 1) * 512],
                        in_=ob[:, (mch - 1) * 512:(mch + 1) * 512])

        for i in range(NT):
            proj(i)
            if i % 4 == 0:
                ab_chunk(i // 4)
            mth(i)
            if i > 0:
                attn(i - 1)
        attn(NT - 1)

    nc.compile()
    return nc


_NC_CACHE = None


def kernel(**inputs):
    global _NC_CACHE
    import ml_dtypes
    bf = ml_dtypes.bfloat16

    x = np.asarray(inputs["x"])
    Wq = np.asarray(inputs["Wq"]); Wk = np.asarray(inputs["Wk"])
    Wv = np.asarray(inputs["Wv"]); Wo = np.asarray(inputs["Wo"])
    pol_dir = np.asarray(inputs["pol_dir"]); pol_WA = np.asarray(inputs["pol_WA"])
    pol_WB = np.asarray(inputs["pol_WB"]); pol_gate = np.asarray(inputs["pol_gate"])
    gtp_gamma = np.asarray(inputs["gtp_gamma"])
    assert x.shape == (1, T, D_MODEL)

    pol = np.clip(pol_dir.astype(np.float64), -1.0, 1.0)
    gamma = np.maximum(np.log1p(np.exp(gtp_gamma.astype(np.float64))), 1e-6)
    c_h = (pol / float(MAX_SEQ_LEN) + gamma)                      # [16]
    gate = 1.0 / (1.0 + np.exp(-pol_gate.astype(np.float64)))     # [16]

    # xt[p, d*T + t] = x[t, 128d + p]
    xt = np.ascontiguousarray(
        x[0].T.reshape(ND, P, T).transpose(1, 0, 2).reshape(P, ND * T)
    ).astype(bf)

    # RoPE tables in [t(part), tile, d]: cos[t, d]; sin'[t, d] has the
    # rotate-half sign baked in (negative for d < 64).
    inv_freq = 1.0 / (ROPE_BASE ** (np.arange(0, D_HEAD, 2, dtype=np.float64) / D_HEAD))
    tpos = np.arange(T, dtype=np.float64)
    ang = tpos[:, None] * inv_freq[None, :]                       # [T, 64]
    cos_td = np.concatenate([np.cos(ang), np.cos(ang)], 1)        # [T, 128]
    sin_td = np.concatenate([-np.sin(ang), np.sin(ang)], 1)
    cosd = cos_td.reshape(NT, P, P).transpose(1, 0, 2).reshape(P, NT * P).astype(bf)
    sind = sin_td.reshape(NT, P, P).transpose(1, 0, 2).reshape(P, NT * P).astype(bf)

    pp = np.arange(P, dtype=np.float64)

    if _NC_CACHE is None:
        _NC_CACHE = _build_kernel()

    in_maps = []
    for c in range(N_CORES):
        heads = [2 * c, 2 * c + 1]
        kv = c // 2
        # wqkv[p, d*512 + [q0|q1|k|v]]
        wq_c = Wq[:, heads[0] * D_HEAD:(heads[0] + 1) * D_HEAD]
        wq_c2 = Wq[:, heads[1] * D_HEAD:(heads[1] + 1) * D_HEAD]
        wk_c = Wk[:, kv * D_HEAD:(kv + 1) * D_HEAD]
        wv_c = Wv[:, kv * D_HEAD:(kv + 1) * D_HEAD]
        wqkv_full = np.concatenate([wq_c, wq_c2, wk_c, wv_c], axis=1)  # [2048, 512]
        wqkv_h = wqkv_full.reshape(ND, P, 512).transpose(1, 0, 2).reshape(P, ND * 512)
        wab_full = np.concatenate([pol_WA, pol_WB], axis=1)            # [2048, 64]
        wab_h = wab_full.reshape(ND, P, 2 * RANK).transpose(1, 0, 2).reshape(P, -1)
        # wo[p, h*2048 + m] = Wo[head*128 + p, m]
        wo_h = np.concatenate(
            [Wo[h * D_HEAD:(h + 1) * D_HEAD, :] for h in heads], axis=1)
        # fh = 1 + 2/g;  c1be[p, s, b] = (g/2)*exp(-128c(15-b))*exp(c(p-127))
        # (the last factor is the per-key exp bias, folded into the table)
        fh_h = np.zeros((P, HPC), dtype=np.float64)
        c1b_h = np.zeros((P, HPC, NT), dtype=np.float64)
        for s, h in enumerate(heads):
            bb = np.arange(NT, dtype=np.float64)
            fh_h[:, s] = 1.0 + 2.0 / gate[h]
            c1b_h[:, s, :] = ((gate[h] / 2.0)
                              * np.exp(-128.0 * c_h[h] * (15.0 - bb[None, :]))
                              * np.exp(c_h[h] * (pp[:, None] - 127.0)))
        in_maps.append({
            "xt": xt,
            "wqkv": np.ascontiguousarray(wqkv_h).astype(bf),
            "wab": np.ascontiguousarray(wab_h).astype(bf),
            "cosd": cosd, "sind": sind,
            "wo": np.ascontiguousarray(wo_h).astype(bf),
            "fh": fh_h.astype(np.float32),
            "c1b": np.ascontiguousarray(c1b_h.reshape(P, HPC * NT)).astype(bf),
        })

    from concourse.bass_utils import run_bass_kernel_spmd
    res = run_bass_kernel_spmd(_NC_CACHE, in_maps, core_ids=list(range(N_CORES)))
    import sys as _sys
    _sys.modules[__name__]._LAST_RES = res
    total = np.zeros((T, D_MODEL), dtype=np.float32)
    for c in range(N_CORES):
        total += res.results[c]["out"].astype(np.float32)
    return total[None, :, :]


# revision 53
# speedup vs baseline: 1.0096x; 1.0023x over previous
"""MicrotubuleAttention TRN2 kernel, head-sharded across 8 NeuronCores.

Core c computes q-heads {2c, 2c+1} with kv-head c//2 and its slice of the
output projection; the host sums the 8 partial projections.

Key structure (single activation-table kernel):
 - Projections in [t, d] layout; RoPE via free-dim half-shifts; Q/K/A-out
   transposed to [d, t] on the PE for the attention matmuls.
 - Scores computed TRANSPOSED per 128-block: S^T[tk, tq] so exp() emits P^T
   directly (no per-block transpose of the softmax matrix).
 - Bias (gate*sigmoid(lowrank) + c_h*(j - row_max)) is prefilled into PSUM by
   the DVE before the QK matmul accumulates on top; sigmoid is computed as
   0.5*tanh(m/2)+0.5 so tanh/exp share one activation table (no reloads).
 - The shifted c_h*(j - 128i - 127) bias keeps scores <= ~7, so softmax runs
   without a max pass; the row sum comes from a ones-column appended to V and
   normalization uses the DVE fast reciprocal.
"""
import numpy as np

D_MODEL = 2048
N_HEADS = 16
D_HEAD = 128
MAX_SEQ_LEN = 4096
RANK = 32
ROPE_BASE = 10000.0
T = 2048
N_CORES = 8
HPC = 2                  # q heads per core
P = 128
NT = T // P              # 16 row tiles
ND = D_MODEL // P        # 16 d-model chunks
SD = float(np.sqrt(D_HEAD))
CH = 8                   # j-blocks per PSUM chunk (8*128 = 1024 cols)
NEG = -1.0e30


def _build_kernel():
    import concourse.bass as bass
    import concourse.mybir as mybir
    import concourse.tile as tile
    from concourse import bacc
    from concourse.masks import make_identity
    from contextlib import ExitStack

    f32 = mybir.dt.float32
    bf16 = mybir.dt.bfloat16
    AF = mybir.ActivationFunctionType
    ALU = mybir.AluOpType

    nc = bacc.Bacc("TRN2", target_bir_lowering=False, debug=False,
                   num_devices=N_CORES)

    xt = nc.dram_tensor("xt", [P, ND * T], bf16, kind="ExternalInput")
    wqkv = nc.dram_tensor("wqkv", [P, ND * 512], bf16, kind="ExternalInput")
    wab = nc.dram_tensor("wab", [P, ND * 2 * RANK], bf16, kind="ExternalInput")
    cosd = nc.dram_tensor("cosd", [P, NT * P], bf16, kind="ExternalInput")
    sind = nc.dram_tensor("sind", [P, NT * P], bf16, kind="ExternalInput")
    wo = nc.dram_tensor("wo", [P, HPC * D_MODEL], bf16, kind="ExternalInput")
    fh = nc.dram_tensor("fh", [P, HPC], f32, kind="ExternalInput")
    c1b = nc.dram_tensor("c1b", [P, HPC * NT], bf16, kind="ExternalInput")
    out = nc.dram_tensor("out", [T, D_MODEL], bf16, kind="ExternalOutput")

    with tile.TileContext(nc) as tc, ExitStack() as ctx:
        con = ctx.enter_context(tc.tile_pool(name="con", bufs=1))
        work = ctx.enter_context(tc.tile_pool(name="work", bufs=3))
        wk3 = ctx.enter_context(tc.tile_pool(name="wk3", bufs=4))
        small = ctx.enter_context(tc.tile_pool(name="small", bufs=4))
        ps_pq = ctx.enter_context(tc.tile_pool(name="ps_pq", bufs=1, space="PSUM"))
        ps_ms = ctx.enter_context(tc.tile_pool(name="ps_ms", bufs=2, space="PSUM"))
        ps_ot = ctx.enter_context(tc.tile_pool(name="ps_ot", bufs=1, space="PSUM"))
        ps_tp = ctx.enter_context(tc.tile_pool(name="ps_tp", bufs=1, space="PSUM"))
        ps_op = ctx.enter_context(tc.tile_pool(name="ps_op", bufs=1, space="PSUM"))

        # ---- constants / inputs (DMA order = criticality) ----
        # wqkv + the first 512 t-cols of xt stream in d-chunk groups so the
        # first projection chain starts within a few us.
        wqkv_sb = con.tile([P, ND, 512], bf16)
        xt_sb = con.tile([P, ND, T], bf16)
        wab_sb = con.tile([P, ND, 2 * RANK], bf16)
        nc.sync.dma_start(out=wab_sb[:], in_=wab[:, :])
        for g in range(4):
            dsl = slice(g * 4, (g + 1) * 4)
            nc.sync.dma_start(out=wqkv_sb[:, dsl],
                              in_=wqkv[:, g * 4 * 512:(g + 1) * 4 * 512])
            xs = xt[:, 0:1]
            src = bass.AP(tensor=xs.tensor, offset=xs.offset + g * 4 * T,
                          ap=[xs.ap[0], [T, 4], [1, 512]])
            nc.sync.dma_start(out=xt_sb[:, dsl, 0:512], in_=src)
        cos_sb = con.tile([P, NT, P], bf16)
        nc.sync.dma_start(out=cos_sb[:], in_=cosd[:, :])
        sin_sb = con.tile([P, NT, P], bf16)
        nc.sync.dma_start(out=sin_sb[:], in_=sind[:, :])
        fh_sb = con.tile([P, HPC], f32)
        nc.sync.dma_start(out=fh_sb[:], in_=fh[:, :])
        c1b_sb = con.tile([P, HPC, NT], bf16)
        nc.sync.dma_start(out=c1b_sb[:], in_=c1b[:, :])
        wo_sb = con.tile([P, HPC, D_MODEL], bf16)
        for g in range(3):
            c0 = 512 * (g + 1)
            xs = xt[:, 0:1]
            src = bass.AP(tensor=xs.tensor, offset=xs.offset + c0,
                          ap=[xs.ap[0], [T, ND], [1, 512]])
            nc.sync.dma_start(out=xt_sb[:, :, c0:c0 + 512], in_=src)
            if g == 0:
                nc.sync.dma_start(out=wo_sb[:], in_=wo[:, :])
        ident = con.tile([P, P], bf16)
        make_identity(nc, ident)

        qkt = con.tile([P, 3, T], bf16)          # q0^T q1^T k^T in [d, t]
        v_sb = con.tile([P, NT, 132], bf16)      # V [tk, d] + ones col @128
        abt_sb = con.tile([2 * RANK, T], bf16)   # [A^T; B^T] rows 0:32 / 32:64
        bt_sb = con.tile([RANK, T], bf16)        # B^T relocated to partition 0
        nc.vector.memset(v_sb[:, :, 128:129], 1.0)

        def view3(ap, n, w):
            """reshape trailing free dim of a 2-D AP slice into [n, w]"""
            return bass.AP(tensor=ap.tensor, offset=ap.offset,
                           ap=[ap.ap[0], [ap.ap[1][0] * w, n],
                               [ap.ap[1][0], w]])

        def bcast(ap, w):
            """[P, n] AP -> [P, n, w] with stride-0 last dim"""
            return bass.AP(tensor=ap.tensor, offset=ap.offset,
                           ap=[ap.ap[0], ap.ap[1], [0, w]])

        def bcast_mid(ap, n):
            """[P, w] AP -> [P, n, w] with stride-0 middle dim"""
            return bass.AP(tensor=ap.tensor, offset=ap.offset,
                           ap=[ap.ap[0], [0, n], ap.ap[1]])

        def ab_chunk(c):
            csl = slice(c * 512, (c + 1) * 512)
            pa = ps_pq.tile([P, 512], mybir.dt.float32, tag="pq")
            for d in range(ND):
                nc.tensor.matmul(pa[0:2 * RANK, :], wab_sb[:, d],
                                 xt_sb[:, d, csl],
                                 start=(d == 0), stop=(d == ND - 1))
            nc.scalar.copy(abt_sb[:, csl], pa[0:2 * RANK, :])
            # relocate B^T rows to partition base 0 (matmul operands must share
            # a base partition; engines can't shift partitions, DMA can)
            nc.sync.dma_start(out=bt_sb[:, csl], in_=abt_sb[RANK:2 * RANK, csl])

        def proj(i):
            tsl = slice(i * P, (i + 1) * P)
            pq = ps_pq.tile([P, 512], mybir.dt.float32, tag="pq")
            for d in range(ND):
                nc.tensor.matmul(pq[:], xt_sb[:, d, tsl], wqkv_sb[:, d],
                                 start=(d == 0), stop=(d == ND - 1))
            qkc = work.tile([P, 4, P], bf16, tag="qkc")
            nc.scalar.copy(qkc[:], pq[:])
            nc.gpsimd.tensor_copy(v_sb[:, i, 0:128], qkc[:, 3])
            # RoPE in [t, d]: half-shifted reads, sign baked into sin table
            ro = work.tile([P, 3, P], bf16, tag="ro")
            nc.vector.tensor_tensor(
                ro[:, :, 0:64], qkc[:, 0:3, 64:128],
                bcast_mid(sin_sb[:, i, 0:64], 3), op=ALU.mult)
            nc.vector.tensor_tensor(
                ro[:, :, 64:128], qkc[:, 0:3, 0:64],
                bcast_mid(sin_sb[:, i, 64:128], 3), op=ALU.mult)
            qc = work.tile([P, 3, P], bf16, tag="qc")
            nc.vector.tensor_tensor(
                qc[:], qkc[:, 0:3], bcast_mid(cos_sb[:, i, :], 3), op=ALU.mult)
            roped = work.tile([P, 3, P], bf16, tag="roped")
            nc.gpsimd.tensor_tensor(roped[:], qc[:], ro[:], op=ALU.add)
            tp_ = ps_tp.tile([P, 384], bf16, tag="tp")
            for hh in range(3):
                nc.tensor.transpose(tp_[:, hh * P:(hh + 1) * P],
                                    roped[:, hh], ident[:])
            nc.scalar.copy(qkt[:, :, tsl], tp_[:])

        def sview(flat_ap, h, nb):
            """[128, 1024] psum slice -> [128, nb, 128] picking head h out of
            interleaved 256-wide blocks"""
            base = flat_ap  # AP over sp_[:, h*128 : h*128+128]
            return bass.AP(tensor=base.tensor, offset=base.offset,
                           ap=[base.ap[0], [256, nb], base.ap[1]])

        th_tiles = {}

        def mth(i):
            tsl = slice(i * P, (i + 1) * P)
            nb_all = i + 1
            th = work.tile([P, NT, P], bf16, tag="th")
            th_tiles[i] = th
            for j0 in range(0, nb_all, CH):
                nb = min(CH, nb_all - j0)
                mp_ = ps_ms.tile([P, CH * P], mybir.dt.float32, tag="ms")
                for jj in range(nb):
                    j = j0 + jj
                    nc.tensor.matmul(mp_[:, jj * P:(jj + 1) * P],
                                     bt_sb[:, j * P:(j + 1) * P],
                                     abt_sb[0:RANK, tsl], start=True, stop=True,
                                     skip_group_check=True)
                nc.scalar.activation(th[:, j0:j0 + nb], mp_[:, 0:nb * P],
                                     AF.Tanh, scale=0.5)

        def attn(i):
            tsl = slice(i * P, (i + 1) * P)
            nb_all = i + 1
            th = th_tiles.pop(i)
            # merged-head scores: one matmul per (i, j) block, rhs = [q0|q1];
            # pm keeps the head-interleaved layout [tk_p, j, h, tq] so one exp
            # instruction covers both heads (per-partition bias lives in c1be)
            pm = work.tile([P, NT, HPC, P], bf16, tag="pm")
            for j0 in range(0, nb_all, 4):
                nb = min(4, nb_all - j0)
                sp_ = ps_ms.tile([P, CH * P], mybir.dt.float32, tag="ms")
                for jj in range(nb):
                    j = j0 + jj
                    nc.tensor.matmul(sp_[:, jj * 256:(jj + 1) * 256],
                                     qkt[:, 2, j * P:(j + 1) * P],
                                     qkt[:, 0:2, tsl],
                                     start=True, stop=True,
                                     skip_group_check=True)
                nc.scalar.activation(pm[:, j0:j0 + nb], sp_[:, 0:nb * 256],
                                     AF.Exp, scale=1.0 / SD)
            aoutT = wk3.tile([P, HPC, P], bf16, tag="aoutT")
            b0 = NT - 1 - i
            for h in range(HPC):
                pmh = bass.AP(tensor=pm[:].tensor,
                              offset=pm[:].offset + h * P,
                              ap=[pm[:].ap[0], [HPC * P, nb_all], [1, P]])
                # fold gate*sigmoid + per-block decay + exp bias into P^T:
                #   pm *= c1be[p, b] * (th + 1 + 2/g)
                # (chunked on the last tiles so AV can start early in the tail)
                FCH = CH
                tmp = work.tile([P, NT, P], bf16, tag="ftmp")
                for f0 in range(0, nb_all, FCH):
                    fn = min(FCH, nb_all - f0)
                    nc.vector.tensor_scalar_add(tmp[:, f0:f0 + fn],
                                                th[:, f0:f0 + fn],
                                                fh_sb[:, h:h + 1])
                    nc.vector.tensor_tensor(
                        tmp[:, f0:f0 + fn], tmp[:, f0:f0 + fn],
                        bcast(c1b_sb[:, h, b0 + f0:b0 + f0 + fn], P),
                        op=ALU.mult)
                    nc.vector.tensor_tensor(pm[:, f0:f0 + fn, h],
                                            pm[:, f0:f0 + fn, h],
                                            tmp[:, f0:f0 + fn], op=ALU.mult)
                # causal mask on the diagonal block: keep tk(p) <= tq(f)
                nc.gpsimd.affine_select(out=pm[:, i, h], in_=pm[:, i, h],
                                        pattern=[[1, P]], compare_op=ALU.is_ge,
                                        fill=0.0, base=0, channel_multiplier=-1)
                ot_ = ps_ot.tile([P, 132], mybir.dt.float32, tag="ot")
                for j in range(nb_all):
                    nc.tensor.matmul(ot_[:, 0:129], pm[:, j, h],
                                     v_sb[:, j, 0:129],
                                     start=(j == 0), stop=(j == nb_all - 1))
                rec = small.tile([P, 1], mybir.dt.float32, tag="rec")
                nc.vector.reciprocal_approx_fast(out=rec[:], in_=ot_[:, 128:129])
                aout = work.tile([P, P], bf16, tag="aout")
                nc.vector.tensor_scalar_mul(aout[:], ot_[:, 0:128], rec[:])
                tp2 = ps_tp.tile([P, 384], bf16, tag="tp")
                nc.tensor.transpose(tp2[:, 0:128], aout[:], ident[:])
                nc.scalar.copy(aoutT[:, h], tp2[:, 0:128])
            # output projection for this row tile
            ob = work.tile([P, D_MODEL], bf16, tag="ob")
            for mch in range(4):
                po = ps_op.tile([P, 512], mybir.dt.float32, tag="po")
                for h in range(HPC):
                    nc.tensor.matmul(po[:], aoutT[:, h],
                                     wo_sb[:, h, mch * 512:(mch + 1) * 512],
                                     start=(h == 0), stop=(h == HPC - 1))
                if mch % 2 == 0:
                    nc.scalar.copy(ob[:, mch * 512:(mch + 1) * 512], po[:])
                else:
                    nc.vector.tensor_copy(ob[:, mch * 512:(mch + 1) * 512],
                                          po[:])
                nc.sync.dma_start(
                    out=out[tsl, mch * 512:(mch + 1) * 512],
                    in_=ob[:, mch * 512:(mch + 1) * 512])

        for i in range(NT):
            proj(i)
            if i % 4 == 0:
                ab_chunk(i // 4)
            mth(i)
            if i > 0:
                attn(i - 1)
        attn(NT - 1)

    nc.compile()
    return nc


_NC_CACHE = None


def kernel(**inputs):
    global _NC_CACHE
    import ml_dtypes
    bf = ml_dtypes.bfloat16

    x = np.asarray(inputs["x"])
    Wq = np.asarray(inputs["Wq"]); Wk = np.asarray(inputs["Wk"])
    Wv = np.asarray(inputs["Wv"]); Wo = np.asarray(inputs["Wo"])
    pol_dir = np.asarray(inputs["pol_dir"]); pol_WA = np.asarray(inputs["pol_WA"])
    pol_WB = np.asarray(inputs["pol_WB"]); pol_gate = np.asarray(inputs["pol_gate"])
    gtp_gamma = np.asarray(inputs["gtp_gamma"])
    assert x.shape == (1, T, D_MODEL)

    pol = np.clip(pol_dir.astype(np.float64), -1.0, 1.0)
    gamma = np.maximum(np.log1p(np.exp(gtp_gamma.astype(np.float64))), 1e-6)
    c_h = (pol / float(MAX_SEQ_LEN) + gamma)                      # [16]
    gate = 1.0 / (1.0 + np.exp(-pol_gate.astype(np.float64)))     # [16]

    # xt[p, d*T + t] = x[t, 128d + p]
    xt = np.ascontiguousarray(
        x[0].T.reshape(ND, P, T).transpose(1, 0, 2).reshape(P, ND * T)
    ).astype(bf)

    # RoPE tables in [t(part), tile, d]: cos[t, d]; sin'[t, d] has the
    # rotate-half sign baked in (negative for d < 64).
    inv_freq = 1.0 / (ROPE_BASE ** (np.arange(0, D_HEAD, 2, dtype=np.float64) / D_HEAD))
    tpos = np.arange(T, dtype=np.float64)
    ang = tpos[:, None] * inv_freq[None, :]                       # [T, 64]
    cos_td = np.concatenate([np.cos(ang), np.cos(ang)], 1)        # [T, 128]
    sin_td = np.concatenate([-np.sin(ang), np.sin(ang)], 1)
    cosd = cos_td.reshape(NT, P, P).transpose(1, 0, 2).reshape(P, NT * P).astype(bf)
    sind = sin_td.reshape(NT, P, P).transpose(1, 0, 2).reshape(P, NT * P).astype(bf)

    pp = np.arange(P, dtype=np.float64)

    if _NC_CACHE is None:
        _NC_CACHE = _build_kernel()

    in_maps = []
    for c in range(N_CORES):
        heads = [2 * c, 2 * c + 1]
        kv = c // 2
        # wqkv[p, d*512 + [q0|q1|k|v]]
        wq_c = Wq[:, heads[0] * D_HEAD:(heads[0] + 1) * D_HEAD]
        wq_c2 = Wq[:, heads[1] * D_HEAD:(heads[1] + 1) * D_HEAD]
        wk_c = Wk[:, kv * D_HEAD:(kv + 1) * D_HEAD]
        wv_c = Wv[:, kv * D_HEAD:(kv + 1) * D_HEAD]
        wqkv_full = np.concatenate([wq_c, wq_c2, wk_c, wv_c], axis=1)  # [2048, 512]
        wqkv_h = wqkv_full.reshape(ND, P, 512).transpose(1, 0, 2).reshape(P, ND * 512)
        wab_full = np.concatenate([pol_WA, pol_WB], axis=1)            # [2048, 64]
        wab_h = wab_full.reshape(ND, P, 2 * RANK).transpose(1, 0, 2).reshape(P, -1)
        # wo[p, h*2048 + m] = Wo[head*128 + p, m]
        wo_h = np.concatenate(
            [Wo[h * D_HEAD:(h + 1) * D_HEAD, :] for h in heads], axis=1)
        # fh = 1 + 2/g;  c1be[p, s, b] = (g/2)*exp(-128c(15-b))*exp(c(p-127))
        # (the last factor is the per-key exp bias, folded into the table)
        fh_h = np.zeros((P, HPC), dtype=np.float64)
        c1b_h = np.zeros((P, HPC, NT), dtype=np.float64)
        for s, h in enumerate(heads):
            bb = np.arange(NT, dtype=np.float64)
            fh_h[:, s] = 1.0 + 2.0 / gate[h]
            c1b_h[:, s, :] = ((gate[h] / 2.0)
                              * np.exp(-128.0 * c_h[h] * (15.0 - bb[None, :]))
                              * np.exp(c_h[h] * (pp[:, None] - 127.0)))
        in_maps.append({
            "xt": xt,
            "wqkv": np.ascontiguousarray(wqkv_h).astype(bf),
            "wab": np.ascontiguousarray(wab_h).astype(bf),
            "cosd": cosd, "sind": sind,
            "wo": np.ascontiguousarray(wo_h).astype(bf),
            "fh": fh_h.astype(np.float32),
            "c1b": np.ascontiguousarray(c1b_h.reshape(P, HPC * NT)).astype(bf),
        })

    from concourse.bass_utils import run_bass_kernel_spmd
    res = run_bass_kernel_spmd(_NC_CACHE, in_maps, core_ids=list(range(N_CORES)))
    import sys as _sys
    _sys.modules[__name__]._LAST_RES = res
    total = np.zeros((T, D_MODEL), dtype=np.float32)
    for c in range(N_CORES):
        total += res.results[c]["out"].astype(np.float32)
    return total[None, :, :]
